# revision 10
# baseline (speedup 1.0000x reference)
"""Trainium2 Bass kernel for nn_MixedHeadsV2 (mixed-head causal attention).

Full inputs in, full output out. Sharding: 8 cores = 4 batches x 2 head-groups.
Each core handles one batch and 4 of the 8 base heads: even cores heads
{0,1,4,5}, odd cores {2,3,6,7}. Heads 0-3 ("heavy") have effective head size
128; heads 4-7 ("light") have effective head size 64 (their mixed weight rows
64:128 are exactly zero), so the two light heads are packed into one 128-wide
tensor for projections and run concurrently on disjoint PE row groups in
attention.

Per-core pipeline (all on one NeuronCore, Tile-scheduled):
  1. Build effective-weight mixing patterns effA/effB from `weights` via tiny
     rank-1 K=1 matmuls against memset masks (bf16).
  2. x -> bf16 -> DRAM scratch -> DMA-transpose to x^T (xbar, frees PE).
  3. W = base * eff (DVE), PE-transpose to W^T (bf16).
  4. Projections q^T,k^T (d-major; PSUM->SBUF copies on ScalarE, idle in this
     phase) and v (t-major, ones column fused for the softmax denominator).
  5. Causal attention, scoresT layout [s128, t512]: scores = k^T.T @ q^T into
     3-bank PSUM groups, exp on ACT (scale folded; no max-subtraction:
     |scaled scores| < 3), causal masking of diagonal tiles via precomputed
     affine_select masks, AV with fused row-sum (M=d+1), normalize on DVE.
"""
import sys

for p in ("/opt/trn_rl_repo",):
    if p not in sys.path:
        sys.path.append(p)

import numpy as np

import concourse.bass as bass
import concourse.tile as tile
from concourse import bacc, mybir
from concourse.bass_utils import run_bass_kernel_spmd

FP32 = mybir.dt.float32
BF16 = mybir.dt.bfloat16
AF = mybir.ActivationFunctionType
ALU = mybir.AluOpType

T = 2048
C = 512
HS = 128          # heavy head size (= padded head size)
NT128 = T // 128  # 16
NT512 = T // 512  # 4
NCC = C // 128    # 4
SCALE = float(1.0 / np.sqrt(128.0))
SGRP = 3          # score tiles (512 wide) per exp group; 3 banks * 2 bufs + 2 = 8

_CACHE = {}


def _build():
    nc = bacc.Bacc("TRN2", target_bir_lowering=False, debug=False, num_devices=8)
    x_d = nc.dram_tensor("x", [T, C], FP32, kind="ExternalInput")
    w_d = nc.dram_tensor("w", [4, 1], FP32, kind="ExternalInput")
    bq_d = nc.dram_tensor("bq", [4, HS, C], FP32, kind="ExternalInput")
    bk_d = nc.dram_tensor("bk", [4, HS, C], FP32, kind="ExternalInput")
    bv_d = nc.dram_tensor("bv", [4, HS, C], FP32, kind="ExternalInput")
    out_d = nc.dram_tensor("out", [T, 4 * HS], FP32, kind="ExternalOutput")

    with tile.TileContext(nc) as tc:
        _emit(nc, tc, x_d, w_d, bq_d, bk_d, bv_d, out_d)
    nc.compile()
    return nc


def _emit(nc, tc, x_d, w_d, bq_d, bk_d, bv_d, out_d):
    from contextlib import ExitStack

    xbf_d = nc.dram_tensor("xbf_scratch", [T, C], BF16)

    ctx = ExitStack()
    prep_ctx = ExitStack()
    with ctx:
        # ---- persistent SBUF pools ----
        const_p = ctx.enter_context(tc.tile_pool(name="const", bufs=1))
        wts_p = ctx.enter_context(tc.tile_pool(name="wts", bufs=1))
        xt_p = ctx.enter_context(tc.tile_pool(name="xt", bufs=1))
        qk_p = ctx.enter_context(tc.tile_pool(name="qk", bufs=1))
        v_p = ctx.enter_context(tc.tile_pool(name="v", bufs=1))
        pt_p = ctx.enter_context(tc.tile_pool(name="pt", bufs=1))
        o_p = ctx.enter_context(tc.tile_pool(name="o", bufs=3))
        r_p = ctx.enter_context(tc.tile_pool(name="r", bufs=3))
        # ---- PSUM pools: 3-bank score groups x2 + 1-bank small x2 = 8 banks
        sps = ctx.enter_context(tc.tile_pool(name="sps", bufs=2, space="PSUM"))
        ps = ctx.enter_context(tc.tile_pool(name="ps", bufs=2, space="PSUM"))
        stage_p = prep_ctx.enter_context(tc.tile_pool(name="stage", bufs=3))

        # ================= constants =================
        ones_b = const_p.tile([128, C], BF16, tag="ones_b")
        nc.vector.memset(ones_b[:], 1.0)
        ident_b = const_p.tile([128, 128], BF16, tag="ident_b")
        nc.gpsimd.affine_select(
            ident_b[:], ones_b[:, 0:128], pattern=[[1, 128]],
            compare_op=ALU.is_equal, fill=0.0, base=0, channel_multiplier=-1)
        # causal triangle for the diagonal 128x128 block: tri[s, t] = (t >= s)
        tri = const_p.tile([128, 128], BF16, tag="tri")
        nc.gpsimd.affine_select(
            tri[:], ones_b[:, 0:128], pattern=[[1, 128]],
            compare_op=ALU.is_ge, fill=0.0, base=0, channel_multiplier=-1)

        # ================= eff patterns (bf16 rank-1 matmuls) ============
        # effA[d, e] = sum_i w_i * (d < hs_i) * (e < emb_i)         (heads 0-3)
        # effB[d, e] = same for i in {1,3} with (d%64 < hs_i)       (packed light)
        HSL = (64, 32, 128, 64)
        EMB = (256, 256, 512, 512)
        wsc = []
        for i in range(4):
            wi = const_p.tile([1, 1], FP32, name=f"wsc{i}", tag=f"wsc{i}")
            nc.scalar.dma_start(wi[:], w_d.ap()[i:i + 1, :])
            wsc.append(wi)
        effA = const_p.tile([128, C], FP32, tag="effA")
        effB = const_p.tile([128, C], FP32, tag="effB")
        for eff, cfgs, ext in ((effA, (0, 1, 2, 3), False), (effB, (1, 3), True)):
            p = ps.tile([128, 512], FP32, tag="ps")
            for n, i in enumerate(cfgs):
                u = stage_p.tile([1, 128], BF16, name=f"u{i}{ext}", tag="u_row")
                nc.vector.memset(u[:], 0.0)
                if ext:  # packed light: both 64-halves get the (d%64 < hs) pattern
                    nc.vector.memset(u[0:1, 0:min(HSL[i], 64)], 1.0)
                    nc.vector.memset(u[0:1, 64:64 + min(HSL[i], 64)], 1.0)
                else:
                    nc.vector.memset(u[0:1, 0:HSL[i]], 1.0)
                uw = stage_p.tile([1, 128], BF16, name=f"uw{i}{ext}", tag="uw_row")
                nc.vector.tensor_scalar_mul(uw[:], u[:], wsc[i][:])
                vrow = stage_p.tile([1, C], BF16, name=f"v{i}{ext}", tag="v_row")
                nc.vector.memset(vrow[:], 0.0)
                nc.vector.memset(vrow[0:1, 0:EMB[i]], 1.0)
                nc.tensor.matmul(p[:], uw[:], vrow[:],
                                 start=(n == 0), stop=(n == len(cfgs) - 1))
            nc.vector.tensor_copy(eff[:], p[:])

        # ================= x -> bf16 -> x^T via DMA transpose ============
        # 4 x 1MB loads ([512,512] viewed as [128, 4*512]), cast, 4 stores,
        # then xbar transposes; loads and transposes share the sync HWDGE
        # ring in that order so loads are never stuck behind a transpose.
        xt = [xt_p.tile([128, T], BF16, name=f"xt{cc}", tag=f"xt{cc}")
              for cc in range(NCC)]
        xbv = x_d.ap().rearrange("(q a p) c -> q p a c", q=4, p=128)
        xbfv = xbf_d.ap().rearrange("(q a p) c -> q p a c", q=4, p=128)
        for q in range(4):
            xs = stage_p.tile([128, 4 * C], FP32, name="xs", tag="xs", bufs=2)
            nc.sync.dma_start(xs[:].rearrange("p (a c) -> p a c", a=4), xbv[q])
            xb = stage_p.tile([128, 4 * C], BF16, name="xb", tag="xb", bufs=2)
            nc.vector.tensor_copy(xb[:], xs[:])
            nc.scalar.dma_start(xbfv[q], xb[:].rearrange("p (a c) -> p a c", a=4))
        for half in range(2):
            for cc in range(NCC):
                nc.sync.dma_start_transpose(
                    xt[cc][:, half * 1024:(half + 1) * 1024],
                    xbf_d.ap()[half * 1024:(half + 1) * 1024,
                               cc * 128:(cc + 1) * 128])

        # ================= effective weights, transposed =================
        # wt[j][cc]: [128c, 128d] bf16, j in 0..8 (q h0,h1,l | k ... | v ...)
        wt = [[wts_p.tile([128, 128], BF16, name=f"wt{j}_{cc}", tag=f"wt{j}_{cc}")
               for cc in range(NCC)] for j in range(9)]
        for pi, bd in enumerate((bq_d, bk_d, bv_d)):
            for hj in range(3):
                j = pi * 3 + hj
                w_bf = stage_p.tile([128, C], BF16, tag="w_bf")
                base = stage_p.tile([128, C], FP32, tag="base")
                if hj < 2:
                    nc.scalar.dma_start(base[:], bd.ap()[hj])
                    nc.vector.tensor_mul(w_bf[:], base[:], effA[:])
                else:
                    nc.scalar.dma_start(base[0:64, :], bd.ap()[2][0:64, :])
                    nc.scalar.dma_start(base[64:128, :], bd.ap()[3][0:64, :])
                    nc.vector.tensor_mul(w_bf[:], base[:], effB[:])
                for cc in range(NCC):
                    pt = ps.tile([128, 512], BF16, tag="ps")
                    nc.tensor.transpose(
                        pt[:, 0:128], w_bf[:, cc * 128:(cc + 1) * 128], ident_b[:])
                    nc.vector.tensor_copy(wt[j][cc][:], pt[:, 0:128])

        # ================= projections =================
        # q^T, k^T: [128d, T] bf16 per tensor (heavy0, heavy1, packed light)
        qt = [qk_p.tile([128, T], BF16, name=f"qt{h}", tag=f"qt{h}") for h in range(3)]
        # kt: heavy0, heavy1, l0 (rows 64:128 zero), l1 (rows 0:64 zero).
        # Zero-padded splits keep light scores as plain K=128 matmuls.
        kt = [qk_p.tile([128, T], BF16, name=f"kt{h}", tag=f"kt{h}") for h in range(4)]
        nc.vector.memset(kt[2][64:128, :], 0.0)
        nc.vector.memset(kt[3][0:64, :], 0.0)
        for dst, j0 in ((qt, 0), (kt, 3)):
            for hj in range(3):
                for tj in range(NT512):
                    p = ps.tile([128, 512], FP32, tag="ps")
                    for cc in range(NCC):
                        nc.tensor.matmul(
                            p[:], wt[j0 + hj][cc][:],
                            xt[cc][:, tj * 512:(tj + 1) * 512],
                            start=(cc == 0), stop=(cc == NCC - 1))
                    sl = slice(tj * 512, (tj + 1) * 512)
                    if dst is kt and hj == 2:
                        nc.vector.tensor_copy(kt[2][0:64, sl], p[0:64, :])
                        nc.vector.tensor_copy(kt[3][64:128, sl], p[64:128, :])
                    else:
                        nc.vector.tensor_copy(dst[hj][:, sl], p[:])
        # v tiles: heavy [128, 129] (ones col at 128); light packed [128, 130]
        # (head l0 cols 0:64 + ones at 64, head l1 cols 65:129 + ones at 129)
        vtiles = [[v_p.tile([128, 132], BF16, name=f"v{h}_{i}", tag=f"v{h}_{i}")
                   for i in range(NT128)] for h in range(3)]
        for hj in range(3):
            for i in range(NT128):
                p = ps.tile([128, 512], FP32, tag="ps")
                for cc in range(NCC):
                    nc.tensor.matmul(
                        p[:, 0:128], xt[cc][:, i * 128:(i + 1) * 128],
                        wt[6 + hj][cc][:],
                        start=(cc == 0), stop=(cc == NCC - 1))
                vt_i = vtiles[hj][i]
                if hj < 2:
                    nc.vector.tensor_copy(vt_i[:, 0:128], p[:, 0:128])
                    nc.vector.memset(vt_i[:, 128:129], 1.0)
                else:
                    nc.vector.tensor_copy(vt_i[:, 0:64], p[:, 0:64])
                    nc.vector.tensor_copy(vt_i[:, 65:129], p[:, 64:128])
                    nc.vector.memset(vt_i[:, 64:65], 1.0)
                    nc.vector.memset(vt_i[:, 129:130], 1.0)

        prep_ctx.close()

        # ================= attention =================
        # (kt idx, qt idx, v idx, v_lo, v_hi, out col); light heads use the
        # zero-padded kt splits so every unit is a plain K=128 pipeline.
        ATT = [
            (0, 0, 0, 0, 129, 0),
            (1, 1, 1, 0, 129, 128),
            (2, 2, 2, 0, 65, 256),
            (3, 2, 2, 65, 130, 384),
        ]
        for (ktj, qtj, vj, v_lo, v_hi, ocol) in ATT:
            for tj in range(NT512):
                S = 4 * tj + 4  # s-chunks (128 wide) needed for this t-chunk
                ptile = pt_p.tile([128, S * 512], BF16,
                                  name=f"pt_{tj}", tag=f"pt_{tj}",
                                  bufs=2)
                g = 0
                while g < S:
                    gw = min(SGRP, S - g)
                    sp = sps.tile([128, SGRP * 512], FP32, name="sp", tag="sps")
                    for k in range(gw):
                        i = g + k
                        nc.tensor.matmul(
                            sp[:, k * 512:(k + 1) * 512],
                            kt[ktj][:, i * 128:(i + 1) * 128],
                            qt[qtj][:, tj * 512:(tj + 1) * 512],
                            start=True, stop=True)
                    nc.scalar.activation(
                        ptile[:, g * 512:(g + gw) * 512],
                        sp[:, 0:gw * 512], AF.Exp, scale=SCALE)
                    g += gw
                # causal triangle on the 4 diagonal 128x128 blocks
                for r in range(4):
                    i = 4 * tj + r
                    blk = slice(i * 512 + r * 128, i * 512 + (r + 1) * 128)
                    nc.vector.tensor_mul(ptile[:, blk], ptile[:, blk], tri[:])
                # AV + fused row-sum, normalize, store
                w = v_hi - v_lo
                for m in range(4):
                    ti = 4 * tj + m  # global t128 index
                    op = ps.tile([128, 512], FP32, name="op", tag="ps")
                    for i in range(ti + 1):
                        nc.tensor.matmul(
                            op[:, 0:w],
                            ptile[:, i * 512 + m * 128: i * 512 + (m + 1) * 128],
                            vtiles[vj][i][:, v_lo:v_hi],
                            start=(i == 0), stop=(i == ti))
                    rec = r_p.tile([128, 1], FP32, name="rec", tag="rec")
                    nc.vector.reciprocal(rec[:], op[:, w - 1:w])
                    ob = o_p.tile([128, 128], FP32, name="ob", tag="ob")
                    nc.vector.tensor_scalar_mul(
                        ob[:, 0:w - 1], op[:, 0:w - 1], rec[:])
                    nc.sync.dma_start(
                        out_d.ap()[ti * 128:(ti + 1) * 128,
                                   ocol:ocol + (w - 1)],
                        ob[:, 0:w - 1])


def _shard_inputs(x, weights, base_K, base_Q, base_V):
    in_maps = []
    for c in range(8):
        b = c // 2
        hsel = [0, 1, 4, 5] if c % 2 == 0 else [2, 3, 6, 7]
        in_maps.append({
            "x": np.ascontiguousarray(x[b]),
            "w": np.ascontiguousarray(weights.reshape(4, 1)),
            "bq": np.ascontiguousarray(base_Q[hsel]),
            "bk": np.ascontiguousarray(base_K[hsel]),
            "bv": np.ascontiguousarray(base_V[hsel]),
        })
    return in_maps


def _gather(results):
    out = np.zeros((4, T, 8 * HS), np.float32)
    for c in range(8):
        o = results[c]["out"]
        hsel = [0, 1, 4, 5] if c % 2 == 0 else [2, 3, 6, 7]
        for j, h in enumerate(hsel):
            out[c // 2][:, h * HS:(h + 1) * HS] = o[:, j * HS:(j + 1) * HS]
    return out


def get_nc():
    if "nc" not in _CACHE:
        _CACHE["nc"] = _build()
    return _CACHE["nc"]


def kernel(x, weights, base_K, base_Q, base_V):
    x = np.asarray(x, np.float32)
    weights = np.asarray(weights, np.float32)
    base_K = np.asarray(base_K, np.float32)
    base_Q = np.asarray(base_Q, np.float32)
    base_V = np.asarray(base_V, np.float32)
    nc = get_nc()
    in_maps = _shard_inputs(x, weights, base_K, base_Q, base_V)
    res = run_bass_kernel_spmd(nc, in_maps, core_ids=list(range(8)))
    return _gather(res.results)
